# revision 9
# baseline (speedup 1.0000x reference)
"""Trainium2 Bass kernel for nn_CLIP_3v3d_brats (dense_cnn head + gated 1x1 conv).

Sharding: 8 cores = batch(2) x 4 D-slabs of `pred`. Core r handles batch
b=r//4, d-planes [24*dq, 24*dq+24). The dominant einsum
logits[b,k,:] = sum_c effw[b,k,c]*pred[b,c,:] runs as a block-diagonal
bf16 matmul (G=4 position groups on partitions -> K=128, M=12).

NO COLLECTIVES: each core computes the full GAP head for its own batch
(x_e[b] is replicated to the 4 cores of that batch), eliminating the
~100us collective cold-start + wait the old channel-sharded head paid.

DMA: everything streams in bf16 (host casts; rel-err budget is 2e-2,
bf16 costs ~3e-3). Lines are fat (27.6KB for pred tiles) and every big
tensor is split across BOTH hardware DGE queues (sync + scalar engines)
so ~32 descriptors are in flight instead of 16 -- the old kernel was
capped at ~230GB/s by one queue with one descriptor per DMA engine.

Head: GroupNorm stats via ACT(sum)+DVE(sumsq with accum), then
u = max(x + t, 0) with t = beta'/s (s = rsqrt(var)*gamma > 0), computed
in-place split ACT/GPSIMD. The conv+global-mean collapses to 27 window
sums; these are computed with a separable 3-stage tensor_reduce cascade
on DVE (k, then j, then i) over an h-parity-split layout so the
innermost reduce dim is stride-1: 21.6k cols total instead of 35.9k.
The per-channel scale s folds into the S->bf16 cast, and x_feat comes
from 54 accumulating 128x128 matmuls against W2d^T/1331.
"""
import sys
import types

sys.path.insert(0, "/opt/trn_rl_repo")

import numpy as np
import ml_dtypes

# Register the NTFF profile hook the agent image's antenv lacks (only
# needed when TRACE is enabled; harmless otherwise).
try:
    import antenv.axon_hooks  # noqa: F401
except ImportError:
    try:
        import trn_agent_boot.trn_boot as _tb

        _hooks = types.ModuleType("antenv.axon_hooks")
        _the_hook = _tb._ntff_profile_via_ctypes("/opt/axon/libaxon_pjrt.so")
        _hooks.get_axon_ntff_profile_hook = lambda: _the_hook
        _hooks.set_axon_ntff_profile_hook = lambda h: None
        sys.modules["antenv.axon_hooks"] = _hooks
    except Exception:
        pass

from concourse import bacc, tile, mybir
from concourse.bass_utils import run_bass_kernel_spmd

f32 = mybir.dt.float32
bf16 = mybir.dt.bfloat16
AF = mybir.ActivationFunctionType
ALU = mybir.AluOpType
AX = mybir.AxisListType
BF = ml_dtypes.bfloat16

N_CORES = 8
B = 2
K = 3
G = 4
EPS = 1e-5
SC = 55296                 # stream cols per core (221184 positions / G)
NT = 4                     # pred tiles
TC = SC // NT              # 13824 cols per pred tile
NMM_T = TC // 512          # 27 matmuls per tile
OC = 4608                  # out chunk cols (9 matmuls)
NOC = SC // OC             # 12 out chunks
XC = 13824                 # x_e cols per core (24^3)
XH = XC // 2               # per x_e tile (12 d-planes)
NELEM = 8 * XC             # elements per GroupNorm group (8ch x 24^3)

TRACE = False
LAST_EXEC_NS = None
_CACHE = {}


def _build_program(tneg_safe=False):
    nc = bacc.Bacc("TRN2", target_bir_lowering=False, debug=False,
                   num_devices=N_CORES)

    def din(name, shape, dt=f32):
        return nc.dram_tensor(name, shape, dt, kind="ExternalInput").ap()

    pred_d = din("pred_s", [128, SC], bf16)
    xe_d = din("xe_s", [128, XC], bf16)
    w2dt_d = din("w2dt", [128, 27 * 256], bf16)
    gmask_d = din("gmask", [128, 16])
    gexpT_d = din("gexpT", [16, 128])
    gnw_d = din("gnw", [128, 2])
    gapbT_d = din("gapbT", [128, 2])
    w_cfT_d = din("w_cfT", [128, 2 * 512], bf16)
    bcf3_d = din("bcf3", [3, 4 * 128], bf16)
    id3_d = din("id3", [3, 3], bf16)
    w_cT_d = din("w_cT", [128, 4 * 256], bf16)
    bcT_d = din("bcT", [128, 2])
    w_a1T_d = din("w_a1T", [128, 2 * 16], bf16)
    ba1_d = din("ba1", [16, 1])
    w_a2T_d = din("w_a2T", [16, 32], bf16)
    ba2_d = din("ba2", [32, 1])
    wseg3_d = din("wseg3", [32, 3], bf16)
    bseg12_d = din("bseg12", [12, 1])

    out_d = nc.dram_tensor("out_s", [12, SC], bf16,
                           kind="ExternalOutput").ap()

    with tile.TileContext(nc) as tc:
        with tc.tile_pool(name="pred", bufs=NT) as pp, \
             tc.tile_pool(name="xe", bufs=2) as xp, \
             tc.tile_pool(name="junk", bufs=2) as jp, \
             tc.tile_pool(name="small", bufs=1) as sp, \
             tc.tile_pool(name="outp", bufs=2) as op, \
             tc.tile_pool(name="sps", bufs=4, space="PSUM") as sps:

            # ---- DMA: x_e first (head critical path), halves on the two
            # hardware DGE queues (sync + scalar engines); then pred tiles.
            xts = []
            for i in range(2):
                xt = xp.tile([128, XH], bf16, tag="xe")
                nc.sync.dma_start(xt[0:64, :], xe_d[0:64, i * XH:(i + 1) * XH])
                nc.scalar.dma_start(xt[64:128, :],
                                    xe_d[64:128, i * XH:(i + 1) * XH])
                xts.append(xt)
            pts = []
            for t in range(NT):
                pt = pp.tile([128, TC], bf16, tag="pt")
                nc.sync.dma_start(pt[0:64, :], pred_d[0:64, t * TC:(t + 1) * TC])
                nc.scalar.dma_start(pt[64:128, :],
                                    pred_d[64:128, t * TC:(t + 1) * TC])
                pts.append(pt)

            # small/weight loads on the gpsimd queue (idle early)
            def gload(name, shape, dram, dt=f32):
                t_ = sp.tile(shape, dt, name=name)
                nc.gpsimd.dma_start(t_[:], dram[:])
                return t_

            w2dt = gload("w2dt", [128, 27 * 256], w2dt_d, bf16)
            gmask = gload("gmask", [128, 16], gmask_d)
            gexpT = gload("gexpT", [16, 128], gexpT_d)
            gnw = gload("gnw", [128, 2], gnw_d)
            gapbT = gload("gapbT", [128, 2], gapbT_d)
            w_cfT = gload("w_cfT", [128, 2 * 512], w_cfT_d, bf16)
            bcf3 = gload("bcf3", [3, 4 * 128], bcf3_d, bf16)
            id3 = gload("id3", [3, 3], id3_d, bf16)
            w_cT = gload("w_cT", [128, 4 * 256], w_cT_d, bf16)
            bcT = gload("bcT", [128, 2], bcT_d)
            w_a1T = gload("w_a1T", [128, 2 * 16], w_a1T_d, bf16)
            ba1 = gload("ba1", [16, 1], ba1_d)
            w_a2T = gload("w_a2T", [16, 32], w_a2T_d, bf16)
            ba2 = gload("ba2", [32, 1], ba2_d)
            wseg3 = gload("wseg3", [32, 3], wseg3_d, bf16)
            bseg12 = gload("bseg12", [12, 1], bseg12_d)

            with tc.tile_pool(name="hps", bufs=4, space="PSUM") as hps:
                # ---- GroupNorm stats: per-channel sum (ACT) + sumsq (DVE),
                # interleaved so the two engines run concurrently.
                stat8 = sp.tile([128, 8], f32)
                for i in range(2):
                    for h in range(2):
                        sl = xts[i][:, h * (XH // 2):(h + 1) * (XH // 2)]
                        c = 2 * i + h
                        jk = jp.tile([128, XH // 2], bf16, tag="jk")
                        nc.scalar.activation(jk[:], sl, AF.Copy,
                                             accum_out=stat8[:, c:c + 1])
                        jk2 = jp.tile([128, XH // 2], bf16, tag="jk")
                        nc.vector.tensor_tensor_reduce(
                            jk2[:], sl, sl, 1.0, 0.0, ALU.mult, ALU.add,
                            accum_out=stat8[:, 4 + c:5 + c])

                # pairwise adds -> sq [128, 2] = (sum_x, sum_x2)
                s4 = sp.tile([128, 4], f32)
                nc.vector.tensor_add(s4[:], stat8[:, 0:8:2], stat8[:, 1:8:2])
                sq = sp.tile([128, 2], f32)
                nc.vector.tensor_add(sq[:], s4[:, 0:4:2], s4[:, 1:4:2])

                # group stats via mask matmul -> [16, 2]
                # (head PSUM tiles are all one bank-shaped tag, sliced)
                gs_t = hps.tile([128, 512], f32, tag="hp")
                gs_ps = gs_t[0:16, 0:2]
                nc.tensor.matmul(gs_ps, gmask[:], sq[:], start=True,
                                 stop=True)
                me = sp.tile([128, 2], f32)
                nc.vector.tensor_scalar_mul(me[0:16, :], gs_ps,
                                            1.0 / NELEM)   # (mu, ex2)
                musq = sp.tile([16, 1], f32)
                nc.vector.tensor_mul(musq[:], me[0:16, 0:1], me[0:16, 0:1])
                var = sp.tile([16, 1], f32)
                nc.vector.tensor_sub(var[:], me[0:16, 1:2], musq[:])
                vare = sp.tile([16, 1], f32)
                nc.vector.tensor_scalar_add(vare[:], var[:], float(EPS))
                sd = sp.tile([16, 1], f32)
                nc.scalar.activation(sd[:], vare[:], AF.Sqrt)
                murs = sp.tile([16, 2], f32)
                nc.vector.tensor_copy(murs[:, 0:1], me[0:16, 0:1])
                nc.vector.reciprocal(murs[:, 1:2], sd[:])

                # expand to channels: chmr [128, 2] = (mu_c, rs_c)
                ch_t = hps.tile([128, 512], f32, tag="hp")
                ch_ps = ch_t[:, 0:2]
                nc.tensor.matmul(ch_ps, gexpT[:], murs[:], start=True,
                                 stop=True)
                chmr = sp.tile([128, 2], f32)
                nc.vector.tensor_copy(chmr[:], ch_ps)
                # s_c = rs*gamma ; b_c = beta - mu*s ; t_c = b_c / s_c
                s_c = sp.tile([128, 1], f32)
                nc.vector.tensor_mul(s_c[:], chmr[:, 1:2], gnw[:, 0:1])
                mus = sp.tile([128, 1], f32)
                nc.vector.tensor_mul(mus[:], chmr[:, 0:1], s_c[:])
                b_c = sp.tile([128, 1], f32)
                nc.vector.tensor_sub(b_c[:], gnw[:, 1:2], mus[:])
                if not tneg_safe:
                    rs_s = sp.tile([128, 1], f32)
                    nc.vector.reciprocal(rs_s[:], s_c[:])
                    t_c = sp.tile([128, 1], f32)
                    nc.vector.tensor_mul(t_c[:], b_c[:], rs_s[:])

                # ---- u = max(x + t, 0) in place; ACT does the lower half,
                # GPSIMD the upper (tensor_scalar add/max). In safe mode
                # (gamma may be <= 0) ACT does everything as relu(s*x+b).
                for i in range(2):
                    lo = xts[i][:, 0:XH // 2]
                    hi = xts[i][:, XH // 2:XH]
                    if tneg_safe:
                        nc.scalar.activation(lo, lo, AF.Relu, bias=b_c[:, 0:1],
                                             scale=s_c[:, 0:1])
                        nc.scalar.activation(hi, hi, AF.Relu, bias=b_c[:, 0:1],
                                             scale=s_c[:, 0:1])
                    else:
                        nc.scalar.activation(lo, lo, AF.Relu, bias=t_c[:, 0:1])
                        nc.gpsimd.tensor_scalar(hi, hi, t_c[:, 0:1], 0.0,
                                                ALU.add, ALU.max)

                # ---- separable window sums (DVE tensor_reduce cascade) ----
                # A[c, d, w, kh] = sum_k u[c, d, w, h=kh+2k]
                A = sp.tile([128, 24 * 24 * 3], f32)
                A3 = A[:].rearrange("p (d w k) -> p d w k", d=24, w=24, k=3)
                for i in range(2):
                    xv = xts[i][:].rearrange("p (d w hp hh) -> p d w hp hh",
                                             d=12, w=24, hp=2, hh=12)
                    dl = slice(12 * i, 12 * i + 12)
                    for kh, (hp, h0) in enumerate([(0, 0), (1, 0), (0, 1)]):
                        nc.vector.tensor_reduce(
                            A3[:, dl, :, kh:kh + 1],
                            xv[:, :, :, hp, h0:h0 + 11],
                            AX.X, ALU.add)
                # B[c, d, kw, kh] = sum_j A[c, d, w=kw+2j, kh]
                Bt = sp.tile([128, 24 * 9], f32)
                B3 = Bt[:].rearrange("p (d q) -> p d q", d=24, q=9)
                for kw in range(3):
                    nc.vector.tensor_reduce(
                        B3[:, :, 3 * kw:3 * kw + 3],
                        A3[:, :, kw:kw + 21:2, :].transpose([0, 1, 3, 2]),
                        AX.X, ALU.add)
                # S[c, (kd kw kh)] = sum_i B[c, d=kd+2i, kw, kh]
                S = sp.tile([128, 27], f32)
                for kd in range(3):
                    nc.vector.tensor_reduce(
                        S[:, 9 * kd:9 * kd + 9],
                        B3[:, kd:kd + 21:2, :].transpose([0, 2, 1]),
                        AX.X, ALU.add)
                # cast to bf16, folding s_c back in (S_true = s_c * S_u)
                Sb = sp.tile([128, 27], bf16)
                if tneg_safe:
                    nc.scalar.activation(Sb[:], S[:], AF.Copy)
                else:
                    nc.scalar.activation(Sb[:], S[:], AF.Copy,
                                         scale=s_c[:, 0:1])

                # ---- x_feat = W2d @ S : 54 accumulating matmuls ----
                xfb = sp.tile([128, 2], f32)
                for ch in range(2):
                    xf_t = hps.tile([128, 512], f32, tag="hp")
                    xf_ps = xf_t[:, 0:1]
                    for o in range(27):
                        nc.tensor.matmul(
                            xf_ps,
                            w2dt[:, o * 256 + ch * 128:o * 256 + ch * 128 + 128],
                            Sb[:, o:o + 1],
                            start=(o == 0), stop=(o == 26))
                    nc.vector.tensor_scalar_add(xfb[:, ch:ch + 1], xf_ps,
                                                gapbT[:, ch:ch + 1])

                # broadcast x_feat over the 3 task cols (bf16)
                ones3 = sp.tile([128, 3], bf16)
                nc.vector.memset(ones3[:], 1.0)
                xcT = sp.tile([128, 6], bf16)
                for pc in range(2):
                    nc.vector.tensor_scalar_mul(xcT[:, pc * 3:pc * 3 + 3],
                                                ones3[:], xfb[:, pc:pc + 1])

                # ---- MLP1: p = relu(Wx@x_feat + (We@emb + b_cf)) ----
                p3T = sp.tile([128, 4 * 3], bf16)
                for oc4 in range(4):
                    p1 = hps.tile([128, 512], f32, name="p1t", tag="hp")[:, 0:3]
                    for pc in range(2):
                        nc.tensor.matmul(
                            p1,
                            w_cfT[:, pc * 512 + oc4 * 128:pc * 512 + oc4 * 128 + 128],
                            xcT[:, pc * 3:pc * 3 + 3],
                            start=(pc == 0), stop=False)
                    nc.tensor.matmul(p1, bcf3[:, oc4 * 128:(oc4 + 1) * 128],
                                     id3[:], start=False, stop=True)
                    nc.scalar.activation(p3T[:, oc4 * 3:oc4 * 3 + 3], p1,
                                         AF.Relu)
                # ---- MLP2: c = W_c p + b_c ----
                c3T = sp.tile([128, 2 * 3], bf16)
                for oc2 in range(2):
                    c1 = hps.tile([128, 512], f32, name="c1t", tag="hp")[:, 0:3]
                    for pc in range(4):
                        nc.tensor.matmul(
                            c1,
                            w_cT[:, pc * 256 + oc2 * 128:pc * 256 + oc2 * 128 + 128],
                            p3T[:, pc * 3:pc * 3 + 3],
                            start=(pc == 0), stop=(pc == 3))
                    nc.scalar.activation(c3T[:, oc2 * 3:oc2 * 3 + 3], c1,
                                         AF.Identity, bias=bcT[:, oc2:oc2 + 1])
                # ---- MLP3 + MLP4 -> gate ----
                h1 = hps.tile([128, 512], f32, name="h1t", tag="hp")[0:16, 0:3]
                for pc in range(2):
                    nc.tensor.matmul(h1, w_a1T[:, pc * 16:pc * 16 + 16],
                                     c3T[:, pc * 3:pc * 3 + 3],
                                     start=(pc == 0), stop=(pc == 1))
                hT = sp.tile([16, 3], bf16)
                nc.scalar.activation(hT[:], h1, AF.Relu, bias=ba1[:, 0:1])
                g1 = hps.tile([128, 512], f32, name="g1t", tag="hp")[0:32, 0:3]
                nc.tensor.matmul(g1, w_a2T[:], hT[:], start=True, stop=True)
                gT = sp.tile([32, 3], bf16)
                nc.scalar.activation(gT[:], g1, AF.Sigmoid, bias=ba2[:, 0:1])

                # effw^T [32c, 3k] and block-diagonal lhsT [128, 12]
                effB = sp.tile([32, 3], bf16)
                nc.vector.tensor_mul(effB[:], gT[:], wseg3[:])
                bd = sp.tile([128, 12], bf16)
                nc.vector.memset(bd[:], 0.0)
                for g in range(G):
                    nc.sync.dma_start(bd[32 * g:32 * g + 32, 3 * g:3 * g + 3],
                                      effB[:])

            # ---- main stream: 108 matmuls, copies round-robin on 3 engines
            ot = None
            for t in range(NT):
                for j in range(NMM_T):
                    gc = t * TC + j * 512          # global stream col
                    if gc % OC == 0:
                        ot = op.tile([12, OC], bf16, tag="ot")
                    po = sps.tile([12, 512], f32, tag="po")
                    nc.tensor.matmul(po[:], bd[:], pts[t][:, j * 512:(j + 1) * 512],
                                     start=True, stop=True)
                    off = gc % OC
                    sl = ot[:, off:off + 512]
                    # GPSIMD cannot read PSUM on TRN2: ACT/DVE only
                    if (t * NMM_T + j) % 2 == 0:
                        nc.scalar.activation(sl, po[:], AF.Identity,
                                             bias=bseg12[:, 0:1])
                    else:
                        nc.vector.tensor_scalar_add(sl, po[:], bseg12[:, 0:1])
                    if gc % OC == OC - 512:
                        oc_i = gc // OC
                        nc.gpsimd.dma_start(out_d[:, oc_i * OC:(oc_i + 1) * OC],
                                            ot[:])

    nc.compile()
    return nc


def _prep_shared(inp):
    """Host-side weight transposes/casts shared by all cores."""
    gn_g = np.asarray(inp["gn_g"], np.float32)
    gn_b = np.asarray(inp["gn_b"], np.float32)
    gap_w = np.asarray(inp["gap_w"], np.float32)
    gap_b = np.asarray(inp["gap_b"], np.float32)
    w_cf = np.asarray(inp["w_cf"], np.float32)
    b_cf = np.asarray(inp["b_cf"], np.float32)
    w_c = np.asarray(inp["w_c"], np.float32)
    b_c = np.asarray(inp["b_c"], np.float32)
    w_a1 = np.asarray(inp["w_a1"], np.float32)
    b_a1 = np.asarray(inp["b_a1"], np.float32)
    w_a2 = np.asarray(inp["w_a2"], np.float32)
    b_a2 = np.asarray(inp["b_a2"], np.float32)
    emb = np.asarray(inp["emb"], np.float32)
    w_seg = np.asarray(inp["w_seg"], np.float32)
    b_seg = np.asarray(inp["b_seg"], np.float32)

    p = np.arange(128)
    gmask = (p[:, None] // 8 == np.arange(16)[None, :]).astype(np.float32)
    gexpT = np.ascontiguousarray(gmask.T)
    gnw = np.stack([gn_g, gn_b], axis=1).astype(np.float32)

    # W2d^T / 1331: [128c, 27o x 256oc]
    w2 = gap_w.reshape(256, 128, 27) / np.float32(1331.0)
    w2dt = np.ascontiguousarray(w2.transpose(1, 2, 0).reshape(128, 27 * 256))

    wx = w_cf[:, 0:256].T                            # [256, 512]
    w_cfT = np.concatenate([wx[128 * pc:128 * (pc + 1), :] for pc in range(2)],
                           axis=1)
    bcf3 = np.ascontiguousarray(b_cf[None, :] + emb @ w_cf[:, 256:512].T)
    id3 = np.eye(3, dtype=np.float32)
    w_cT = np.concatenate([w_c.T[128 * pc:128 * (pc + 1), :] for pc in range(4)],
                          axis=1)
    bcT = np.ascontiguousarray(b_c.reshape(2, 128).T)
    w_a1T = np.concatenate(
        [w_a1.T[128 * pc:128 * (pc + 1), :] for pc in range(2)], axis=1)
    ba1 = b_a1.reshape(16, 1)
    w_a2T = np.ascontiguousarray(w_a2.T)
    ba2 = b_a2.reshape(32, 1)
    wseg3 = np.ascontiguousarray(w_seg.T)
    gapbT = np.ascontiguousarray(gap_b.reshape(2, 128).T)
    bseg12 = np.tile(b_seg, 4).reshape(12, 1)

    sh = dict(gmask=gmask, gexpT=gexpT, gnw=gnw, gapbT=gapbT, bcT=bcT,
              ba1=ba1, ba2=ba2, bseg12=bseg12)
    shb = dict(w2dt=w2dt, w_cfT=w_cfT, bcf3=bcf3, id3=id3, w_cT=w_cT,
               w_a1T=w_a1T, w_a2T=w_a2T, wseg3=wseg3)
    m = {k: np.ascontiguousarray(v, dtype=np.float32) for k, v in sh.items()}
    m.update({k: np.ascontiguousarray(v.astype(BF)) for k, v in shb.items()})
    return m


def kernel(**inputs):
    global LAST_EXEC_NS
    x_e = np.asarray(inputs["x_e"], np.float32)
    pred = np.asarray(inputs["pred"], np.float32)
    gn_g = np.asarray(inputs["gn_g"], np.float32)

    shared = _prep_shared(inputs)
    tneg_safe = bool((gn_g <= 0).any())

    # bf16 cast once, then per-core slicing on the halved data
    pred_b = pred.astype(BF)

    # x_e in h-parity layout per batch: cols = (d, w, hp, hh)
    xe_b = []
    for b in range(B):
        xp_ = x_e[b].reshape(128, 24, 24, 12, 2).transpose(0, 1, 2, 4, 3)
        xe_b.append(np.ascontiguousarray(xp_.reshape(128, XC).astype(BF)))

    in_maps = []
    for r in range(N_CORES):
        b, dq = divmod(r, 4)
        m = dict(shared)
        ps = pred_b[b, :, dq * 24:(dq + 1) * 24].reshape(32, 4, SC)
        m["pred_s"] = np.ascontiguousarray(
            ps.transpose(1, 0, 2).reshape(128, SC))
        m["xe_s"] = xe_b[b]
        in_maps.append(m)

    key = ("nc", tneg_safe)
    if key not in _CACHE:
        _CACHE[key] = _build_program(tneg_safe)
    nc = _CACHE[key]

    res = run_bass_kernel_spmd(nc, in_maps, list(range(N_CORES)),
                               trace=TRACE)
    LAST_EXEC_NS = res.exec_time_ns

    out = np.empty((B, K, 96, 96, 96), np.float32)
    for r in range(N_CORES):
        b, dq = divmod(r, 4)
        o = np.asarray(res.results[r]["out_s"]).astype(np.float32)
        o = o.reshape(4, 3, 6, 96, 96).transpose(1, 0, 2, 3, 4)
        out[b, :, dq * 24:(dq + 1) * 24] = o.reshape(3, 24, 96, 96)
    return out
